# revision 6
# baseline (speedup 1.0000x reference)
"""Additive (Bahdanau) attention on 8 TRN2 NeuronCores, data-parallel over batch.

ctx[b]   = sum_n a[b,n] * V[b,n,:]
a[b,:]   = softmax_n( tanh(h[b]Ww + Wb + V[b]Uw + Ub) @ vw + vb )

Sharding: batch B=128 split 16-per-core across 8 cores; weights replicated.
vb is mathematically irrelevant (softmax shift invariance) but still declared.

Per-core dataflow (per batch b):
  1. gpsimd DMA loads V[b] f32->bf16 (cast in DMA datapath), natural layout.
  2. xbar transpose DMA produces V^T tiles [vdim, n] in SBUF (bf16).
  3. PE: uv^T[a, n] = sum_v Uw[v, a] V^T[v, n]  (64 matmuls into PSUM)
  4. ACT: t^T = tanh(uv^T + bias) with bias = (Ww h[b] + Wb + Ub)^T per partition
  5. PE: e[1, n] = sum_a vw[a] t^T[a, n]
  6. ACT: exps = exp(e) with accum_out = sum (no max subtraction; e is small)
  7. DVE: a = exps / sum ; PE: a^T via K=1 ones-matmul ; cast bf16
  8. PE: ctx[1, :] = a^T . V (natural-layout bf16 copy) ; DMA PSUM->DRAM
"""

import numpy as np

import concourse.bass as bass
import concourse.tile as tile
from concourse import bacc, mybir
from concourse import bass_utils

F32 = mybir.dt.float32
BF16 = mybir.dt.bfloat16
AF = mybir.ActivationFunctionType

B, N, HD, VD, AT = 128, 196, 512, 2048, 512
NCORES = 8
BL = B // NCORES          # 16 batches per core
NJ = VD // 128            # 16 vdim k-tiles
NA = AT // 128            # 4 att tiles
NK = HD // 128            # 4 hdim k-tiles
R1 = N - 128              # 68 rows in second tile
R1P = 80                  # padded row count for xbar transpose (mult of 16)
FT = 128 + R1P            # V^T tile free size (208; cols 196..207 junk)

_TS = bass.ts


def _build_core(tc):
    nc = tc.nc
    h = nc.dram_tensor("h", [BL, HD], F32, kind="ExternalInput").ap()
    V = nc.dram_tensor("V", [BL, N, VD], F32, kind="ExternalInput").ap()
    Ww = nc.dram_tensor("Ww", [HD, AT], F32, kind="ExternalInput").ap()
    Wb = nc.dram_tensor("Wb", [AT], F32, kind="ExternalInput").ap()
    Uw = nc.dram_tensor("Uw", [VD, AT], F32, kind="ExternalInput").ap()
    Ub = nc.dram_tensor("Ub", [AT], F32, kind="ExternalInput").ap()
    vw = nc.dram_tensor("vw", [AT, 1], F32, kind="ExternalInput").ap()
    octx = nc.dram_tensor("out_ctx", [BL, VD], F32, kind="ExternalOutput").ap()
    oa = nc.dram_tensor("out_a", [BL, N], F32, kind="ExternalOutput").ap()

    wpool = tc.alloc_tile_pool(name="weights", bufs=1)
    vpool = tc.alloc_tile_pool(name="vtiles", bufs=3)
    spool = tc.alloc_tile_pool(name="small", bufs=2)
    ppool = tc.alloc_tile_pool(name="ps", bufs=1, space="PSUM")

    # ---------------- preamble: weights / constants ----------------
    uw_sb = wpool.tile([128, NJ, AT], BF16)
    nc.gpsimd.dma_start(uw_sb[:], Uw.rearrange("(j p) a -> p j a", p=128))

    ww_sb = wpool.tile([128, NK, AT], F32)
    nc.sync.dma_start(ww_sb[:], Ww.rearrange("(k p) a -> p k a", p=128))

    h_sb = wpool.tile([1, BL * HD], F32)
    nc.sync.dma_start(
        h_sb[:], h.rearrange("b d -> (b d)").rearrange("(x n) -> x n", x=1))
    wb_sb = wpool.tile([1, AT], F32)
    nc.sync.dma_start(wb_sb[:], Wb.rearrange("(x a) -> x a", x=1))
    ub_sb = wpool.tile([1, AT], F32)
    nc.sync.dma_start(ub_sb[:], Ub.rearrange("(x a) -> x a", x=1))
    vw_sb = wpool.tile([1, AT], F32)
    nc.sync.dma_start(vw_sb[:], vw.rearrange("a x -> x a"))

    ones_f = wpool.tile([1, 1], F32)
    nc.vector.memset(ones_f[:], 1.0)

    # bias sum (Wb + Ub) in row layout, then transpose via K=1 matmul
    bsum = wpool.tile([1, AT], F32)
    nc.vector.tensor_add(bsum[:], wb_sb[:], ub_sb[:])
    bsumT_ps = ppool.tile([128, NA], F32, tag="small", bufs=3)
    for i in range(NA):
        nc.tensor.matmul(bsumT_ps[:, i : i + 1], bsum[0:1, _TS(i, 128)],
                         ones_f[:], start=True, stop=True)
    bsumT_sb = wpool.tile([128, NA], F32)
    nc.vector.tensor_copy(bsumT_sb[:], bsumT_ps[:])

    vwT_ps = ppool.tile([128, NA], F32, tag="small", bufs=3)
    for i in range(NA):
        nc.tensor.matmul(vwT_ps[:, i : i + 1], vw_sb[0:1, _TS(i, 128)],
                         ones_f[:], start=True, stop=True)
    vwT_sb = wpool.tile([128, NA], BF16)
    nc.vector.tensor_copy(vwT_sb[:], vwT_ps[:])

    # h^T tiles: [hdim-part, k, batch] via per-row K=1 matmuls
    hT_sb = wpool.tile([128, NK, BL], F32)
    for k in range(NK):
        hT_ps = ppool.tile([128, BL], F32, tag="small", bufs=3)
        for b in range(BL):
            nc.tensor.matmul(hT_ps[:, b : b + 1],
                             h_sb[0:1, b * HD + k * 128 : b * HD + (k + 1) * 128],
                             ones_f[:], start=True, stop=True)
        nc.vector.tensor_copy(hT_sb[:, k, :], hT_ps[:])

    # whbT[a-part, ai, b] = (Ww^T h)[a, b] + Wb[a] + Ub[a]
    whbT_sb = wpool.tile([128, NA, BL], F32)
    for ai in range(NA):
        whT_ps = ppool.tile([128, BL], F32, tag="small", bufs=3)
        for k in range(NK):
            nc.tensor.matmul(whT_ps[:], ww_sb[:, k, _TS(ai, 128)],
                             hT_sb[:, k, :], start=(k == 0), stop=(k == NK - 1))
        nc.vector.tensor_scalar_add(whbT_sb[:, ai, :], whT_ps[:],
                                    bsumT_sb[:, ai : ai + 1])

    # ---------------- main loop over local batches ----------------
    for b in range(BL):
        vn0 = vpool.tile([128, VD], BF16, tag="vn0")
        nc.gpsimd.dma_start(vn0[:], V[b, 0:128, :])
        vn1 = vpool.tile([128, VD], BF16, tag="vn1")
        nc.vector.memset(vn1[64:R1P, :], 0.0)
        nc.gpsimd.dma_start(vn1[0:R1, :], V[b, 128:N, :])

        vt = vpool.tile([128, NJ, FT], BF16, tag="vt")
        for j in range(NJ):
            nc.sync.dma_start(vt[:, j, 0:128], vn0[:, _TS(j, 128)],
                              transpose=True)
            nc.sync.dma_start(vt[:, j, 128:FT], vn1[0:R1P, _TS(j, 128)],
                              transpose=True)

        # uv^T in PSUM: two tiles of [128, 2, N] (one bank each)
        uvps = []
        tts = []
        for half in range(2):
            uvp = ppool.tile([128, 2, N], F32, tag="uv", bufs=3,
                             name=f"uvp{half}")
            for a2 in range(2):
                ai = half * 2 + a2
                for j in range(NJ):
                    nc.tensor.matmul(uvp[:, a2, :], uw_sb[:, j, _TS(ai, 128)],
                                     vt[:, j, 0:N],
                                     start=(j == 0), stop=(j == NJ - 1))
            tt = vpool.tile([128, 2, N], BF16, tag=f"tt{half}", name=f"tt{half}")
            for a2 in range(2):
                ai = half * 2 + a2
                nc.scalar.activation(tt[:, a2, :], uvp[:, a2, :], AF.Tanh,
                                     bias=whbT_sb[:, ai, b : b + 1])
            uvps.append(uvp)
            tts.append(tt)

        # e[1, N] = vw^T t^T (accumulate over 4 att tiles)
        eps = ppool.tile([1, N], F32, tag="small", bufs=3)
        for half in range(2):
            for a2 in range(2):
                ai = half * 2 + a2
                nc.tensor.matmul(eps[:], vwT_sb[:, ai : ai + 1],
                                 tts[half][:, a2, :],
                                 start=(ai == 0), stop=(ai == NA - 1))

        # softmax over free dim (no max subtraction: |e| is small/bounded)
        exps = spool.tile([1, N], F32, tag="exps")
        ssum = spool.tile([1, 1], F32, tag="ssum")
        nc.scalar.activation(exps[:], eps[:], AF.Exp, accum_out=ssum[:])
        rs = spool.tile([1, 1], F32, tag="rs")
        nc.vector.reciprocal(rs[:], ssum[:])
        a_sb = spool.tile([1, N], F32, tag="a_sb")
        nc.vector.tensor_scalar_mul(a_sb[:], exps[:], rs[:])
        nc.sync.dma_start(oa[b : b + 1, :], a_sb[:])

        # a^T via K=1 ones-matmuls, cast to bf16
        atps = ppool.tile([128, 2], F32, tag="small", bufs=3)
        nc.tensor.matmul(atps[:, 0:1], a_sb[0:1, 0:128], ones_f[:],
                         start=True, stop=True)
        nc.tensor.matmul(atps[0:R1, 1:2], a_sb[0:1, 128:N], ones_f[:],
                         start=True, stop=True)
        at_sb = spool.tile([128, 2], BF16, tag="at_sb")
        nc.vector.tensor_copy(at_sb[:, 0:1], atps[:, 0:1])
        nc.vector.tensor_copy(at_sb[0:R1, 1:2], atps[0:R1, 1:2])

        # ctx[1, VD] = a^T V, chunked by 512 (one PSUM bank per chunk)
        ctx_sb = spool.tile([1, VD], F32, tag="ctx_sb")
        for vc in range(4):
            cps = ppool.tile([1, 512], F32, tag="ctx", bufs=2, name=f"cps{vc}")
            nc.tensor.matmul(cps[:], at_sb[:, 0:1], vn0[:, _TS(vc, 512)],
                             start=True, stop=False)
            nc.tensor.matmul(cps[:], at_sb[0:R1, 1:2], vn1[0:R1, _TS(vc, 512)],
                             start=False, stop=True)
            nc.vector.tensor_copy(ctx_sb[:, _TS(vc, 512)], cps[:])
        nc.sync.dma_start(octx[b : b + 1, :], ctx_sb[:])

    ppool.release()
    spool.release()
    vpool.release()
    wpool.release()


_NC_CACHE = {}


def _get_nc():
    if "nc" not in _NC_CACHE:
        nc = bacc.Bacc("TRN2", target_bir_lowering=False, debug=False)
        with tile.TileContext(nc) as tc:
            _build_core(tc)
        nc.compile()
        _NC_CACHE["nc"] = nc
    return _NC_CACHE["nc"]


def kernel(h, V, Ww, Wb, Uw, Ub, vw, vb, _trace=False):
    h = np.ascontiguousarray(np.asarray(h, dtype=np.float32))
    V = np.ascontiguousarray(np.asarray(V, dtype=np.float32))
    nc = _get_nc()
    in_maps = []
    for c in range(NCORES):
        sl = slice(c * BL, (c + 1) * BL)
        in_maps.append({
            "h": h[sl],
            "V": V[sl],
            "Ww": np.asarray(Ww, np.float32),
            "Wb": np.asarray(Wb, np.float32),
            "Uw": np.asarray(Uw, np.float32),
            "Ub": np.asarray(Ub, np.float32),
            "vw": np.asarray(vw, np.float32),
        })
    res = bass_utils.run_bass_kernel_spmd(
        nc, in_maps, core_ids=list(range(NCORES)), trace=_trace)
    ctx = np.concatenate([res.results[c]["out_ctx"] for c in range(NCORES)], 0)
    a = np.concatenate([res.results[c]["out_a"] for c in range(NCORES)], 0)
    if _trace:
        kernel.last_results = res
    return ctx, a


# revision 13
# speedup vs baseline: 127.2280x; 127.2280x over previous
"""Additive (Bahdanau) attention on 8 TRN2 NeuronCores, data-parallel over batch.

ctx[b]   = sum_n a[b,n] * V[b,n,:]
a[b,:]   = softmax_n( tanh(h[b]Ww + Wb + V[b]Uw + Ub) @ vw + vb )

Sharding: batch B=128 split 16-per-core across 8 cores; weights replicated.
vb is mathematically irrelevant (softmax shift invariance) but still declared.

Per-core dataflow (per batch b):
  1. gpsimd DMA loads V[b] f32->bf16 (cast in DMA datapath), natural layout.
  2. xbar transpose DMA produces V^T tiles [vdim, n] in SBUF (bf16).
  3. PE: uv^T[a, n] = sum_v Uw[v, a] V^T[v, n]  (64 matmuls into PSUM)
  4. ACT: t^T = tanh(uv^T + bias) with bias = (Ww h[b] + Wb + Ub)^T per partition
  5. PE: e[1, n] = sum_a vw[a] t^T[a, n]
  6. ACT: exps = exp(e) with accum_out = sum (no max subtraction; e is small)
  7. DVE: a = exps / sum ; PE: a^T via K=1 ones-matmul ; cast bf16
  8. PE: ctx[1, :] = a^T . V (natural-layout bf16 copy) ; DMA PSUM->DRAM
"""

import numpy as np

import concourse.bass as bass
import concourse.tile as tile
from concourse import bacc, mybir
from concourse import bass_utils

F32 = mybir.dt.float32
BF16 = mybir.dt.bfloat16
AF = mybir.ActivationFunctionType

B, N, HD, VD, AT = 128, 196, 512, 2048, 512
NCORES = 8
BL = B // NCORES          # 16 batches per core
NJ = VD // 128            # 16 vdim k-tiles
NA = AT // 128            # 4 att tiles
NK = HD // 128            # 4 hdim k-tiles
R1 = N - 128              # 68 rows in second tile
R1P = 80                  # padded row count for xbar transpose (mult of 16)
FT = 128 + R1P            # V^T tile free size (208; cols 196..207 junk)

_TS = bass.ts


def _build_core(tc, reps=1):
    nc = tc.nc
    h = nc.dram_tensor("h", [BL, HD], F32, kind="ExternalInput").ap()
    V = nc.dram_tensor("V", [BL, N, VD], F32, kind="ExternalInput").ap()
    Ww = nc.dram_tensor("Ww", [HD, AT], F32, kind="ExternalInput").ap()
    Wb = nc.dram_tensor("Wb", [AT], F32, kind="ExternalInput").ap()
    Uw = nc.dram_tensor("Uw", [VD, AT], F32, kind="ExternalInput").ap()
    Ub = nc.dram_tensor("Ub", [AT], F32, kind="ExternalInput").ap()
    vw = nc.dram_tensor("vw", [AT, 1], F32, kind="ExternalInput").ap()
    octx = nc.dram_tensor("out_ctx", [BL, VD], F32, kind="ExternalOutput").ap()
    oa = nc.dram_tensor("out_a", [BL, N], F32, kind="ExternalOutput").ap()

    wpool = tc.alloc_tile_pool(name="weights", bufs=1)
    ppool = tc.alloc_tile_pool(name="ps", bufs=1, space="PSUM")
    prepool = tc.alloc_tile_pool(name="pre", bufs=1)

    # ---------------- preamble: weights / constants ----------------
    uw_sb = wpool.tile([128, NJ, AT], BF16)
    nc.gpsimd.dma_start(uw_sb[:], Uw.rearrange("(j p) a -> p j a", p=128))

    ww_sb = prepool.tile([128, NK, AT], F32)
    nc.sync.dma_start(ww_sb[:], Ww.rearrange("(k p) a -> p k a", p=128))

    h_sb = prepool.tile([1, BL * HD], F32)
    nc.sync.dma_start(
        h_sb[:], h.rearrange("b d -> (b d)").rearrange("(x n) -> x n", x=1))
    wb_sb = prepool.tile([1, AT], F32)
    nc.sync.dma_start(wb_sb[:], Wb.rearrange("(x a) -> x a", x=1))
    ub_sb = prepool.tile([1, AT], F32)
    nc.sync.dma_start(ub_sb[:], Ub.rearrange("(x a) -> x a", x=1))
    vw_sb = prepool.tile([1, AT], F32)
    nc.sync.dma_start(vw_sb[:], vw.rearrange("a x -> x a"))

    ones_f = wpool.tile([1, 1], F32)
    nc.vector.memset(ones_f[:], 1.0)

    # bias sum (Wb + Ub) in row layout, then transpose via K=1 matmul
    bsum = prepool.tile([1, AT], F32)
    nc.vector.tensor_add(bsum[:], wb_sb[:], ub_sb[:])
    bsumT_ps = ppool.tile([128, NA], F32, tag="small", bufs=2)
    for i in range(NA):
        nc.tensor.matmul(bsumT_ps[:, i : i + 1], bsum[0:1, _TS(i, 128)],
                         ones_f[:], start=True, stop=True)
    bsumT_sb = wpool.tile([128, NA], F32)
    nc.vector.tensor_copy(bsumT_sb[:], bsumT_ps[:])

    vwT_ps = ppool.tile([128, NA], F32, tag="small", bufs=2)
    for i in range(NA):
        nc.tensor.matmul(vwT_ps[:, i : i + 1], vw_sb[0:1, _TS(i, 128)],
                         ones_f[:], start=True, stop=True)
    vwT_sb = wpool.tile([128, NA], BF16)
    nc.vector.tensor_copy(vwT_sb[:], vwT_ps[:])

    # h^T tiles: [hdim-part, k, batch] via per-row K=1 matmuls
    hT_sb = wpool.tile([128, NK, BL], F32)
    for k in range(NK):
        hT_ps = ppool.tile([128, BL], F32, tag="small", bufs=2)
        for b in range(BL):
            nc.tensor.matmul(hT_ps[:, b : b + 1],
                             h_sb[0:1, b * HD + k * 128 : b * HD + (k + 1) * 128],
                             ones_f[:], start=True, stop=True)
        nc.vector.tensor_copy(hT_sb[:, k, :], hT_ps[:])

    # whbT[a-part, ai, b] = (Ww^T h)[a, b] + Wb[a] + Ub[a]
    whbT_sb = wpool.tile([128, NA, BL], F32)
    for ai in range(NA):
        whT_ps = ppool.tile([128, BL], F32, tag="small", bufs=2)
        for k in range(NK):
            nc.tensor.matmul(whT_ps[:], ww_sb[:, k, _TS(ai, 128)],
                             hT_sb[:, k, :], start=(k == 0), stop=(k == NK - 1))
        nc.vector.tensor_scalar_add(whbT_sb[:, ai, :], whT_ps[:],
                                    bsumT_sb[:, ai : ai + 1])

    prepool.release()
    vpool = tc.alloc_tile_pool(name="vtiles", bufs=3)
    spool = tc.alloc_tile_pool(name="small", bufs=2)

    # ---------------- main loop over local batch pairs ----------------
    NP = N + N  # 392: two batches side by side in the free dim
    for g in [gg % (BL // 2) for gg in range((BL // 2) * reps)]:
        b0, b1 = 2 * g, 2 * g + 1
        vns = []
        for bi, b in enumerate((b0, b1)):
            vn0 = vpool.tile([128, VD], BF16, tag=f"vn0_{bi}", name=f"vn0_{bi}")
            nc.gpsimd.dma_start(vn0[:], V[b, 0:128, :])
            vn1 = vpool.tile([128, VD], BF16, tag=f"vn1_{bi}", name=f"vn1_{bi}")
            nc.gpsimd.dma_start(vn1[0:R1, :], V[b, 128:N, :])
            vns.append((vn0, vn1))

        # One xbar-transpose call per source tile: out[p, j, c] = src[c, j*128+p]
        # vt[:, j, 512*bi + c] == V[b]^T[j*128 + p, c]  (c >= N junk per 256-blk)
        vt = vpool.tile([128, NJ, 512], BF16, tag="vt")
        for bi in range(2):
            nc.sync.dma_start_transpose(out=vt[:, :, 256 * bi : 256 * bi + 128],
                                        in_=vns[bi][0][:])
            nc.sync.dma_start_transpose(out=vt[:, :, 256 * bi + 128 : 256 * bi + 256],
                                        in_=vns[bi][1][:])

        # uv^T in PSUM: per att tile [128, 2, N] = both batches (one bank)
        tts = []
        for ai in range(NA):
            uvp = ppool.tile([128, 2, N], F32, tag="uv", bufs=4, name=f"uvp{ai}")
            for j in range(NJ):
                rhs = vt[:, j, :].rearrange("p (s r) -> p s r", s=2)[:, :, 0:N]
                nc.tensor.matmul(uvp[:], uw_sb[:, j, _TS(ai, 128)], rhs,
                                 start=(j == 0), stop=(j == NJ - 1))
            tt = vpool.tile([128, 2, N], BF16, tag=f"tt{ai}", name=f"tt{ai}")
            for bi, b in enumerate((b0, b1)):
                nc.scalar.activation(tt[:, bi, :], uvp[:, bi, :], AF.Tanh,
                                     bias=whbT_sb[:, ai, b : b + 1])
            tts.append(tt)

        # e[1, 2, N] = vw^T t^T (accumulate over 4 att tiles)
        eps = ppool.tile([1, 2, N], F32, tag="small", bufs=2)
        for ai in range(NA):
            nc.tensor.matmul(eps[:], vwT_sb[:, ai : ai + 1], tts[ai][:],
                             start=(ai == 0), stop=(ai == NA - 1))

        # softmax over free dim (no max subtraction: |e| is small/bounded)
        exps = spool.tile([1, 2, N], F32, tag="exps")
        ssum = spool.tile([1, 2], F32, tag="ssum")
        for bi in range(2):
            nc.scalar.activation(exps[:, bi, :], eps[:, bi, :], AF.Exp,
                                 accum_out=ssum[:, bi : bi + 1])
        rs = spool.tile([1, 2], F32, tag="rs")
        nc.vector.reciprocal(rs[:], ssum[:])
        a_sb = spool.tile([1, 2, N], F32, tag="a_sb")
        for bi in range(2):
            nc.vector.tensor_scalar_mul(a_sb[:, bi, :], exps[:, bi, :],
                                        rs[:, bi : bi + 1])
        nc.sync.dma_start(oa[b0 : b0 + 2, :], a_sb[:])

        # a^T via K=1 ones-matmuls, cast to bf16
        atps = ppool.tile([128, 4], F32, tag="small", bufs=2)
        for bi in range(2):
            nc.tensor.matmul(atps[:, 2 * bi : 2 * bi + 1],
                             a_sb[0:1, bi, 0:128], ones_f[:],
                             start=True, stop=True)
            nc.tensor.matmul(atps[0:R1, 2 * bi + 1 : 2 * bi + 2],
                             a_sb[0:1, bi, 128:N], ones_f[:],
                             start=True, stop=True)
        at_sb = spool.tile([128, 4], BF16, tag="at_sb")
        for bi in range(2):
            nc.vector.tensor_copy(at_sb[:, 2 * bi : 2 * bi + 1],
                                  atps[:, 2 * bi : 2 * bi + 1])
            nc.vector.tensor_copy(at_sb[0:R1, 2 * bi + 1 : 2 * bi + 2],
                                  atps[0:R1, 2 * bi + 1 : 2 * bi + 2])

        # ctx[1, VD] = a^T V, chunked by 512 (one PSUM bank per chunk)
        for bi, b in enumerate((b0, b1)):
            vn0, vn1 = vns[bi]
            ctx_sb = spool.tile([1, VD], F32, tag="ctx_sb", name="ctx_sb")
            for vc in range(4):
                cps = ppool.tile([1, 512], F32, tag="ctx", bufs=2,
                                 name=f"cps{vc}")
                nc.tensor.matmul(cps[:], at_sb[:, 2 * bi : 2 * bi + 1],
                                 vn0[:, _TS(vc, 512)], start=True, stop=False)
                nc.tensor.matmul(cps[:], at_sb[0:R1, 2 * bi + 1 : 2 * bi + 2],
                                 vn1[0:R1, _TS(vc, 512)], start=False, stop=True)
                nc.vector.tensor_copy(ctx_sb[:, _TS(vc, 512)], cps[:])
            nc.sync.dma_start(octx[b : b + 1, :], ctx_sb[:])

    spool.release()
    vpool.release()
    ppool.release()
    wpool.release()


_NC_CACHE = {}


def _get_nc(reps=1):
    key = f"nc{reps}"
    if key not in _NC_CACHE:
        nc = bacc.Bacc("TRN2", target_bir_lowering=False, debug=False)
        with tile.TileContext(nc) as tc:
            _build_core(tc, reps=reps)
        nc.compile()
        _NC_CACHE[key] = nc
    return _NC_CACHE[key]


def kernel(h, V, Ww, Wb, Uw, Ub, vw, vb, _trace=False):
    h = np.ascontiguousarray(np.asarray(h, dtype=np.float32))
    V = np.ascontiguousarray(np.asarray(V, dtype=np.float32))
    nc = _get_nc()
    in_maps = []
    for c in range(NCORES):
        sl = slice(c * BL, (c + 1) * BL)
        in_maps.append({
            "h": h[sl],
            "V": V[sl],
            "Ww": np.asarray(Ww, np.float32),
            "Wb": np.asarray(Wb, np.float32),
            "Uw": np.asarray(Uw, np.float32),
            "Ub": np.asarray(Ub, np.float32),
            "vw": np.asarray(vw, np.float32),
        })
    res = bass_utils.run_bass_kernel_spmd(
        nc, in_maps, core_ids=list(range(NCORES)), trace=_trace)
    ctx = np.concatenate([res.results[c]["out_ctx"] for c in range(NCORES)], 0)
    a = np.concatenate([res.results[c]["out_a"] for c in range(NCORES)], 0)
    if _trace:
        kernel.last_results = res
    return ctx, a
